# revision 28
# baseline (speedup 1.0000x reference)
"""Trainium2 Bass kernel for nn_Cross_Attention (B=16, C=256, H=W=96).

reference:
    q = Z1.reshape(B, C, N); k = Zr.reshape(B, C, N)         # N = H*W
    energy    = q @ k^T                                       # [B, C, C]
    attention = softmax(rowmax(energy) - energy, axis=-1)
    out       = attention @ k                                 # [B, C, N]
    return beta * out + Zr
ref absmax ~5.4, tol 2e-2 -> bf16 I/O rounding (~4e-3) is well inside it.

Strategy: data-parallel over batch, 2 batches per NeuronCore on 8 cores.
All HBM I/O is bf16 and minimal: q^T AND k^T are host-packed
partition-major (k^T replaces the natural-Zr upload -- same bytes), so
the N-contraction energy matmul streams BOTH operands straight from
DRAM with no on-chip preprocessing on its critical path.  Natural-layout
k (the mm2 rhs and, via the diagonal fold, the residual) is rebuilt
on-chip by TensorE transpose-mode matmuls whose PSUM->SBUF copies (one
Activation + one DVE copy per group) have an entire phase of slack --
they only feed mm2, ~15us later.  The transpose groups drain lazily,
interleaved a few matmuls at a time into the energy stream.

softmax(max-e) == exp(min-e)/sum(exp(min-e)): only a row-min is needed,
exp args are <= 0, the sum is >= 1.  The residual Zr IS k, so
beta*out + Zr == (beta*attention + I) @ k: beta and 1/sum fold into the
attention weights, I is added to their diagonal block, and mm2 produces
final values directly in PSUM (bitwise bf16(Zr) when beta == 0); its
psum->sbuf downcast copies alternate DVE/Activation.  q^T chunks 2-3 are
ci-split so eng[0] closes ~4us before eng[1], letting each softmax chain
hide under later matmul streams.  Batch b's stores queue on the sync
ring BEHIND batch b+1's loads so the store burst never steals DMA
bandwidth from the loads feeding the PE; the final slab stores in small
back-to-back pieces so DMA init latencies hide.  Dependency-free warm-up
matmuls bridge batch 0's unavoidable wire waits, because a drained PE
restarts at the 0.65GHz p-state for ~3us.
"""

from contextlib import ExitStack

import ml_dtypes
import numpy as np

import concourse.bass as bass
import concourse.tile as tile
from concourse import bacc, mybir
from concourse.bass_utils import run_bass_kernel_spmd
from concourse.masks import make_identity

B, C, H, W = 16, 256, 96, 96
N = H * W                    # 9216
P = 128
NCORES = 8
BL = B // NCORES             # 2 batches per core
CT = C // P                  # 2 c-tiles of 128
NT = N // P                  # 72 contraction tiles for energy
TCH = 18                     # tiles per chunk
NCH = NT // TCH              # 4 chunks (last two ci-split on the q side)
NREG = 2                     # chunks 0..1 are ci-interleaved
TQT = NREG * TCH             # 36 t-tiles in the interleaved qt tensor
NSP = NCH - NREG             # 2 ci-split chunks
NH = N // 2                  # 4608: kb slice width (half a c-tile row)
TPH = NH // P                # 36 n-tiles per h-half
OW = 512                     # mm2 psum chunk width == one full PSUM bank
WPH = NH // OW               # 9 psum chunks per h-half
SW = 3 * OW                  # 1536: store piece width (3 per h-half)

F32 = mybir.dt.float32
BF16 = mybir.dt.bfloat16


def _build_program():
    nc = bacc.Bacc("TRN2", target_bir_lowering=False, debug=False,
                   num_devices=NCORES)

    qt_ext = nc.dram_tensor("qt", [BL, P, TQT, C], BF16, kind="ExternalInput")
    qtt_ext = nc.dram_tensor("qtt", [BL, CT, P, NSP, TCH, P], BF16,
                             kind="ExternalInput")
    kt_ext = nc.dram_tensor("kt", [BL, P, NT, C], BF16, kind="ExternalInput")
    beta_ext = nc.dram_tensor("beta", [1], F32, kind="ExternalInput")
    out_ext = nc.dram_tensor("out", [BL, C, N], BF16, kind="ExternalOutput")

    with tile.TileContext(nc) as tc, ExitStack() as ctx:
        qtp = ctx.enter_context(tc.tile_pool(name="qtp", bufs=4))
        ktp = ctx.enter_context(tc.tile_pool(name="ktp", bufs=5))
        kbp = ctx.enter_context(tc.tile_pool(name="kbp", bufs=6))
        expp = ctx.enter_context(tc.tile_pool(name="expp", bufs=2))
        attp = ctx.enter_context(tc.tile_pool(name="attp", bufs=2))
        atTp = ctx.enter_context(tc.tile_pool(name="atTp", bufs=2))
        outbp = ctx.enter_context(tc.tile_pool(name="outbp", bufs=5))
        statp = ctx.enter_context(tc.tile_pool(name="statp", bufs=8))
        singles = ctx.enter_context(tc.tile_pool(name="singles", bufs=1))
        engp = ctx.enter_context(tc.tile_pool(name="engp", bufs=2, space="PSUM"))
        trp = ctx.enter_context(tc.tile_pool(name="trp", bufs=3, space="PSUM"))
        outp = ctx.enter_context(tc.tile_pool(name="outp", bufs=3, space="PSUM"))

        ident = singles.tile([P, P], BF16)
        beta_sb = singles.tile([P, 1], F32)
        make_identity(nc, ident)
        # after make_identity: the gpsimd ring's ~1us descriptor generation
        # must not sit ahead of the identity fill in the Pool queue
        nc.gpsimd.dma_start(out=beta_sb, in_=beta_ext.ap().to_broadcast((P, 1)))

        # warm-filler sizes for batch 0's wire waits (tuned from traces)
        WARM_HEAD = 28
        WARM_CH1 = 14
        WARM_SP_MM = {0: 12, 18: 21}     # before ch2-ci0 / ch3-ci0 mm streams
        WARM_CI1 = {0: 26, 18: 8}       # before ci1 mm j-indices

        GROUPS = [list(range(g * 4, min(g * 4 + 4, TCH)))
                  for g in range((TCH + 3) // 4)]
        NG = len(GROUPS)

        def warm(n):
            ps = outp.tile([P, OW], F32, name="warm", tag="ps")
            for _ in range(n):
                nc.tensor.matmul(ps[:, :P], lhsT=ident, rhs=ident,
                                 start=True, stop=True)

        def warmb(n):
            # mm2-phase filler: scratch transposes into the (idle) trp pool
            # -- outp rotation is what these fills are bridging, so the
            # energy-phase warm() scratch must not be used here
            trb = trp.tile([P, CT, 4, P], BF16, name="warmb", tag="tr4")
            for _ in range(n):
                nc.tensor.transpose(trb[:, 0, 0, :], ident, ident)

        def emit_tr_group(kt_t, i, g, kb):
            # rebuild natural-k for chunk i, tls GROUPS[g]: transpose each
            # [128n x 128d] sub-tile of the resident k^T chunk, then one
            # Activation copy (dj=0) + one DVE copy (dj=1) into kb
            h = i // 2
            tls = GROUPS[g]
            n = len(tls)
            th0 = i * TCH + tls[0] - h * TPH
            tr = trp.tile([P, CT, 4, P], BF16, name="tr4")
            for j, tl in enumerate(tls):
                for dj in range(CT):
                    nc.tensor.transpose(tr[:, dj, j, :],
                                        kt_t[:, tl, dj * P:(dj + 1) * P],
                                        ident)
            nc.scalar.copy(out=kb[0, h][:, th0 * P:(th0 + n) * P],
                           in_=tr[:, 0, :n, :])
            nc.vector.tensor_copy(out=kb[1, h][:, th0 * P:(th0 + n) * P],
                                  in_=tr[:, 1, :n, :])

        deferred_stores = []
        for b in range(BL):
            eng = [engp.tile([P, C], F32, name="eng") for _ in range(CT)]
            kb = {(cj, hh): kbp.tile([P, NH], BF16, name="kb_t")
                  for hh in range(2) for cj in range(CT)}
            pending = []          # (kt_t, i, g) transpose groups to drain

            def drain(nmax=1):
                for _ in range(min(nmax, len(pending))):
                    emit_tr_group(*pending.pop(0), kb)

            # ---- chunks 0..1, ci-interleaved q; energy matmuls consume
            # q^T and k^T straight from DRAM; previous chunk's transpose
            # groups drain between matmul packs ----
            for i in range(NREG):
                kt_t = ktp.tile([P, TCH, C], BF16, name="kt_t")
                qt_t = qtp.tile([P, TCH, C], BF16, name="qt_t")
                hf = TCH // 2
                # kernel head: extra-fine first pieces so the PE starts
                # its first real matmuls ~1.5us sooner
                pieces = ((0, 3), (3, hf), (hf, TCH)) if i == 0 and b == 0                     else ((0, hf), (hf, TCH))
                for lo, hi in pieces:
                    nc.sync.dma_start(out=kt_t[:, lo:hi, :],
                                      in_=kt_ext[b, :, i * TCH + lo:
                                                 i * TCH + hi, :])
                    nc.sync.dma_start(out=qt_t[:, lo:hi, :],
                                      in_=qt_ext[b, :, i * TCH + lo:
                                                 i * TCH + hi, :])
                if b == 0:
                    warm(WARM_HEAD if i == 0 else WARM_CH1)
                for tl in range(TCH):
                    if tl % 2 == 0:
                        drain(1)
                    t = i * TCH + tl
                    for ci in range(CT):
                        nc.tensor.matmul(
                            eng[ci],
                            lhsT=qt_t[:, tl, ci * P:(ci + 1) * P],
                            rhs=kt_t[:, tl, :],
                            start=(t == 0),
                            stop=False,
                        )
                pending.extend((kt_t, i, g) for g in range(NG))

            # ---- chunks 2..3: q ci-split so eng[0] closes early; k^T
            # chunks interleave with the qtt halves on the wire ----
            kts = []
            qtts = [qtp.tile([P, NSP, TCH, P], BF16, name="qtt_t", tag="qt_t")
                    for _ in range(CT)]
            for isp in range(NSP):
                kt_t = ktp.tile([P, TCH, C], BF16, name="kt_t")
                hf = TCH // 2
                for lo, hi in ((0, hf), (hf, TCH)):
                    nc.sync.dma_start(out=kt_t[:, lo:hi, :],
                                      in_=kt_ext[b, :, (NREG + isp) * TCH + lo:
                                                 (NREG + isp) * TCH + hi, :])
                nc.sync.dma_start(out=qtts[0][:, isp],
                                  in_=qtt_ext[b, 0, :, isp])
                kts.append(kt_t)
            nc.sync.dma_start(out=qtts[1][:, 0], in_=qtt_ext[b, 1, :, 0])
            nc.sync.dma_start(out=qtts[1][:, 1], in_=qtt_ext[b, 1, :, 1])

            # previous batch's stores: behind this batch's loads on the
            # sync ring, so they drain without contending with them
            for dst_ap, src_t in deferred_stores:
                nc.sync.dma_start(out=dst_ap, in_=src_t)
            deferred_stores = []

            for isp in range(NSP):
                for tl in range(TCH):
                    k2 = isp * TCH + tl
                    if b == 0 and k2 in WARM_SP_MM:
                        warm(WARM_SP_MM[k2])
                    if tl % 2 == 0:
                        drain(1)
                    t = (NREG + isp) * TCH + tl
                    nc.tensor.matmul(
                        eng[0],
                        lhsT=qtts[0][:, isp, tl, :],
                        rhs=kts[isp][:, tl, :],
                        start=False,
                        stop=(t == NT - 1),
                    )
                pending.extend((kts[isp], NREG + isp, g) for g in range(NG))

            # ---- softmax(max-e) = exp(min-e)/sum with beta/sum folded in
            # and I added to the diagonal block (residual fold) ----
            attnT = [None] * CT

            def emit_softmax(ci):
                mn = statp.tile([P, 1], F32)
                nc.vector.tensor_reduce(out=mn, in_=eng[ci],
                                        axis=mybir.AxisListType.X,
                                        op=mybir.AluOpType.min)
                ex = expp.tile([P, C], F32)
                sm = statp.tile([P, 1], F32)
                nc.scalar.activation(out=ex, in_=eng[ci],
                                     func=mybir.ActivationFunctionType.Exp,
                                     bias=mn, scale=-1.0, accum_out=sm)
                rc = statp.tile([P, 1], F32)
                nc.vector.reciprocal(out=rc, in_=sm)
                rb = statp.tile([P, 1], F32)
                nc.vector.tensor_mul(out=rb, in0=rc, in1=beta_sb)
                at = attp.tile([P, C], BF16)
                nc.vector.tensor_scalar_mul(out=at, in0=ex, scalar1=rb)
                nc.vector.tensor_add(out=at[:, ci * P:(ci + 1) * P],
                                     in0=at[:, ci * P:(ci + 1) * P],
                                     in1=ident)
                trA = trp.tile([P, CT, P], BF16, name="trA", tag="tr4")
                for dj in range(CT):
                    nc.tensor.transpose(trA[:, dj, :],
                                        at[:, dj * P:(dj + 1) * P], ident)
                atT = atTp.tile([P, CT, P], BF16, name="atT")
                nc.vector.tensor_copy(out=atT, in_=trA)
                attnT[ci] = atT

            # ci=1 energy over chunks 2..3; softmax(0)'s PE transposes slot
            # in at j=32 (~3.4us of chain-hiding cover); remaining natural-k
            # transpose groups drain in the gaps between matmul packs
            ci1_mms = [(isp, tl) for isp in range(NSP) for tl in range(TCH)]
            for j, (isp, tl) in enumerate(ci1_mms):
                if b == 0 and j in WARM_CI1:
                    warm(WARM_CI1[j])
                if j % 2 == 0:
                    drain(1)
                if j == 32:
                    emit_softmax(0)
                t = (NREG + isp) * TCH + tl
                nc.tensor.matmul(
                    eng[1],
                    lhsT=qtts[1][:, isp, tl, :],
                    rhs=kts[isp][:, tl, :],
                    start=False,
                    stop=(t == NT - 1),
                )
            drain(len(pending))

            # ---- out = (beta*A + I) @ k: psum holds the final values;
            # downcast copies alternate DVE/Activation; stores stream per
            # 1536-wide piece;  softmax(1) hides under the ci=0 slabs ----
            def emit_mm2_slab(ci, h2, softmax1_at=None):
                ot = outbp.tile([P, NH], BF16, name="ot")
                for w in range(WPH):
                    if w == softmax1_at:
                        emit_softmax(1)
                    if w == 0:
                        warmb(5)
                    ps = outp.tile([P, OW], F32, name="ps", tag="ps")
                    for dj in range(CT):
                        nc.tensor.matmul(
                            ps,
                            lhsT=attnT[ci][:, dj, :],
                            rhs=kb[dj, h2][:, w * OW:(w + 1) * OW],
                            start=(dj == 0),
                            stop=(dj == CT - 1),
                        )
                    if w % 2 == 0:
                        nc.vector.tensor_copy(
                            out=ot[:, w * OW:(w + 1) * OW], in_=ps)
                    else:
                        nc.scalar.copy(
                            out=ot[:, w * OW:(w + 1) * OW], in_=ps)
                    last_slab = b == BL - 1 and ci == CT - 1 and h2 == 1
                    if last_slab:
                        # 2-chunk pieces at the very end: the store queue
                        # stays non-empty so DMA init latencies hide, and
                        # the final piece is small
                        edges = [2, 4, 6, 7, 8, 9]
                        if w + 1 in edges:
                            lo = ([0] + edges)[edges.index(w + 1)] * OW
                            nc.sync.dma_start(
                                out=out_ext[b, ci * P:(ci + 1) * P,
                                            h2 * NH + lo:
                                            h2 * NH + (w + 1) * OW],
                                in_=ot[:, lo:(w + 1) * OW])
                    elif w % 3 == 2:
                        seg = w // 3
                        dst = out_ext[b, ci * P:(ci + 1) * P,
                                      h2 * NH + seg * SW:
                                      h2 * NH + (seg + 1) * SW]
                        src = ot[:, seg * SW:(seg + 1) * SW]
                        if b < BL - 1:
                            deferred_stores.append((dst, src))
                        else:
                            nc.sync.dma_start(out=dst, in_=src)

            emit_mm2_slab(0, 0)
            emit_mm2_slab(0, 1, softmax1_at=4)
            emit_mm2_slab(1, 0)
            emit_mm2_slab(1, 1)

    nc.compile()
    return nc


_NC_CACHE = None


def _get_program():
    global _NC_CACHE
    if _NC_CACHE is None:
        _NC_CACHE = _build_program()
    return _NC_CACHE


def pack_pm(Z):
    # bf16 partition-major: out[b, p, t, c] = Z[b, c, t*128+p]
    x = Z.reshape(B, C, NT, P).astype(ml_dtypes.bfloat16)
    return x.transpose(0, 3, 2, 1)


def kernel(Z1, Zr, beta):
    Z1 = np.asarray(Z1, dtype=np.float32)
    Zr = np.asarray(Zr, dtype=np.float32)
    beta = np.asarray(beta, dtype=np.float32).reshape(1)

    qt_full = pack_pm(Z1)
    qta = np.ascontiguousarray(qt_full[:, :, :TQT, :])
    qtb = np.ascontiguousarray(
        qt_full[:, :, TQT:, :].reshape(B, P, NSP, TCH, CT, P)
        .transpose(0, 4, 1, 2, 3, 5))
    kt = np.ascontiguousarray(pack_pm(Zr))

    in_maps = []
    for i in range(NCORES):
        s = slice(i * BL, (i + 1) * BL)
        in_maps.append({"qt": qta[s], "qtt": qtb[s], "kt": kt[s],
                        "beta": beta})

    nc = _get_program()
    res = run_bass_kernel_spmd(nc, in_maps, list(range(NCORES)))
    out = np.concatenate([r["out"] for r in res.results], axis=0)
    return out.astype(np.float32).reshape(B, C, H, W)


# revision 29
# speedup vs baseline: 1.0104x; 1.0104x over previous
"""Trainium2 Bass kernel for nn_Cross_Attention (B=16, C=256, H=W=96).

reference:
    q = Z1.reshape(B, C, N); k = Zr.reshape(B, C, N)         # N = H*W
    energy    = q @ k^T                                       # [B, C, C]
    attention = softmax(rowmax(energy) - energy, axis=-1)
    out       = attention @ k                                 # [B, C, N]
    return beta * out + Zr
ref absmax ~5.4, tol 2e-2 -> bf16 I/O rounding (~4e-3) is well inside it.

Strategy: data-parallel over batch, 2 batches per NeuronCore on 8 cores.
All HBM I/O is bf16 and minimal: q^T AND k^T are host-packed
partition-major (k^T replaces the natural-Zr upload -- same bytes), so
the N-contraction energy matmul streams BOTH operands straight from
DRAM with no on-chip preprocessing on its critical path.  Natural-layout
k (the mm2 rhs and, via the diagonal fold, the residual) is rebuilt
on-chip by TensorE transpose-mode matmuls whose PSUM->SBUF copies (one
Activation + one DVE copy per group) have an entire phase of slack --
they only feed mm2, ~15us later.  The transpose groups drain lazily,
interleaved a few matmuls at a time into the energy stream.

softmax(max-e) == exp(min-e)/sum(exp(min-e)): only a row-min is needed,
exp args are <= 0, the sum is >= 1.  The residual Zr IS k, so
beta*out + Zr == (beta*attention + I) @ k: beta and 1/sum fold into the
attention weights, I is added to their diagonal block, and mm2 produces
final values directly in PSUM (bitwise bf16(Zr) when beta == 0); its
psum->sbuf downcast copies alternate DVE/Activation.  q^T chunks 2-3 are
ci-split so eng[0] closes ~4us before eng[1], letting each softmax chain
hide under later matmul streams.  Batch b's stores queue on the sync
ring BEHIND batch b+1's loads so the store burst never steals DMA
bandwidth from the loads feeding the PE; the final slab stores in small
back-to-back pieces so DMA init latencies hide.  Dependency-free warm-up
matmuls bridge batch 0's unavoidable wire waits, because a drained PE
restarts at the 0.65GHz p-state for ~3us.
"""

from contextlib import ExitStack

import ml_dtypes
import numpy as np

import concourse.bass as bass
import concourse.tile as tile
from concourse import bacc, mybir
from concourse.bass_utils import run_bass_kernel_spmd
from concourse.masks import make_identity

B, C, H, W = 16, 256, 96, 96
N = H * W                    # 9216
P = 128
NCORES = 8
BL = B // NCORES             # 2 batches per core
CT = C // P                  # 2 c-tiles of 128
NT = N // P                  # 72 contraction tiles for energy
TCH = 18                     # tiles per chunk
NCH = NT // TCH              # 4 chunks (last two ci-split on the q side)
NREG = 2                     # chunks 0..1 are ci-interleaved
TQT = NREG * TCH             # 36 t-tiles in the interleaved qt tensor
NSP = NCH - NREG             # 2 ci-split chunks
NH = N // 2                  # 4608: kb slice width (half a c-tile row)
TPH = NH // P                # 36 n-tiles per h-half
OW = 512                     # mm2 psum chunk width == one full PSUM bank
WPH = NH // OW               # 9 psum chunks per h-half
SW = 3 * OW                  # 1536: store piece width (3 per h-half)

F32 = mybir.dt.float32
BF16 = mybir.dt.bfloat16


def _build_program():
    nc = bacc.Bacc("TRN2", target_bir_lowering=False, debug=False,
                   num_devices=NCORES)

    qt_ext = nc.dram_tensor("qt", [BL, P, TQT, C], BF16, kind="ExternalInput")
    qtt_ext = nc.dram_tensor("qtt", [BL, CT, P, NSP, TCH, P], BF16,
                             kind="ExternalInput")
    kt_ext = nc.dram_tensor("kt", [BL, P, NT, C], BF16, kind="ExternalInput")
    beta_ext = nc.dram_tensor("beta", [1], F32, kind="ExternalInput")
    out_ext = nc.dram_tensor("out", [BL, C, N], BF16, kind="ExternalOutput")

    with tile.TileContext(nc) as tc, ExitStack() as ctx:
        qtp = ctx.enter_context(tc.tile_pool(name="qtp", bufs=4))
        ktp = ctx.enter_context(tc.tile_pool(name="ktp", bufs=5))
        kbp = ctx.enter_context(tc.tile_pool(name="kbp", bufs=6))
        expp = ctx.enter_context(tc.tile_pool(name="expp", bufs=2))
        attp = ctx.enter_context(tc.tile_pool(name="attp", bufs=2))
        atTp = ctx.enter_context(tc.tile_pool(name="atTp", bufs=2))
        outbp = ctx.enter_context(tc.tile_pool(name="outbp", bufs=5))
        statp = ctx.enter_context(tc.tile_pool(name="statp", bufs=8))
        singles = ctx.enter_context(tc.tile_pool(name="singles", bufs=1))
        engp = ctx.enter_context(tc.tile_pool(name="engp", bufs=2, space="PSUM"))
        trp = ctx.enter_context(tc.tile_pool(name="trp", bufs=3, space="PSUM"))
        outp = ctx.enter_context(tc.tile_pool(name="outp", bufs=3, space="PSUM"))

        ident = singles.tile([P, P], BF16)
        beta_sb = singles.tile([P, 1], F32)
        make_identity(nc, ident)
        # after make_identity: the gpsimd ring's ~1us descriptor generation
        # must not sit ahead of the identity fill in the Pool queue
        nc.gpsimd.dma_start(out=beta_sb, in_=beta_ext.ap().to_broadcast((P, 1)))

        # warm-filler sizes for batch 0's wire waits (tuned from traces)
        WARM_HEAD = 28
        WARM_CH1 = 14
        WARM_SP_MM = {0: 12, 18: 21}     # before ch2-ci0 / ch3-ci0 mm streams
        WARM_CI1 = {0: 26, 18: 8}       # before ci1 mm j-indices

        GROUPS = [list(range(g * 4, min(g * 4 + 4, TCH)))
                  for g in range((TCH + 3) // 4)]
        NG = len(GROUPS)

        def warm(n):
            ps = outp.tile([P, OW], F32, name="warm", tag="ps")
            for _ in range(n):
                nc.tensor.matmul(ps[:, :P], lhsT=ident, rhs=ident,
                                 start=True, stop=True)

        def warmb(n):
            # mm2-phase filler: scratch transposes into the (idle) trp pool
            # -- outp rotation is what these fills are bridging, so the
            # energy-phase warm() scratch must not be used here
            trb = trp.tile([P, CT, 4, P], BF16, name="warmb", tag="tr4")
            for _ in range(n):
                nc.tensor.transpose(trb[:, 0, 0, :], ident, ident)

        def emit_tr_group(kt_t, i, g, kb):
            # rebuild natural-k for chunk i, tls GROUPS[g]: transpose each
            # [128n x 128d] sub-tile of the resident k^T chunk, then one
            # Activation copy (dj=0) + one DVE copy (dj=1) into kb
            h = i // 2
            tls = GROUPS[g]
            n = len(tls)
            th0 = i * TCH + tls[0] - h * TPH
            tr = trp.tile([P, CT, 4, P], BF16, name="tr4")
            for j, tl in enumerate(tls):
                for dj in range(CT):
                    nc.tensor.transpose(tr[:, dj, j, :],
                                        kt_t[:, tl, dj * P:(dj + 1) * P],
                                        ident)
            nc.scalar.copy(out=kb[0, h][:, th0 * P:(th0 + n) * P],
                           in_=tr[:, 0, :n, :])
            nc.vector.tensor_copy(out=kb[1, h][:, th0 * P:(th0 + n) * P],
                                  in_=tr[:, 1, :n, :])

        deferred_stores = []
        for b in range(BL):
            eng = [engp.tile([P, C], F32, name="eng") for _ in range(CT)]
            kb = {(cj, hh): kbp.tile([P, NH], BF16, name="kb_t")
                  for hh in range(2) for cj in range(CT)}
            pending = []          # (kt_t, i, g) transpose groups to drain

            def drain(nmax=1):
                for _ in range(min(nmax, len(pending))):
                    emit_tr_group(*pending.pop(0), kb)

            # ---- chunks 0..1, ci-interleaved q; energy matmuls consume
            # q^T and k^T straight from DRAM; previous chunk's transpose
            # groups drain between matmul packs ----
            for i in range(NREG):
                kt_t = ktp.tile([P, TCH, C], BF16, name="kt_t")
                qt_t = qtp.tile([P, TCH, C], BF16, name="qt_t")
                hf = TCH // 2
                # kernel head: extra-fine first pieces so the PE starts
                # its first real matmuls ~1.5us sooner
                pieces = ((0, 3), (3, hf), (hf, TCH)) if i == 0 and b == 0                     else ((0, hf), (hf, TCH))
                for lo, hi in pieces:
                    nc.sync.dma_start(out=kt_t[:, lo:hi, :],
                                      in_=kt_ext[b, :, i * TCH + lo:
                                                 i * TCH + hi, :])
                    nc.sync.dma_start(out=qt_t[:, lo:hi, :],
                                      in_=qt_ext[b, :, i * TCH + lo:
                                                 i * TCH + hi, :])
                if b == 0:
                    warm(WARM_HEAD if i == 0 else WARM_CH1)
                for tl in range(TCH):
                    if tl % 2 == 0:
                        drain(1)
                    t = i * TCH + tl
                    for ci in range(CT):
                        nc.tensor.matmul(
                            eng[ci],
                            lhsT=qt_t[:, tl, ci * P:(ci + 1) * P],
                            rhs=kt_t[:, tl, :],
                            start=(t == 0),
                            stop=False,
                        )
                pending.extend((kt_t, i, g) for g in range(NG))

            # ---- chunks 2..3: q ci-split so eng[0] closes early; k^T
            # chunks interleave with the qtt halves on the wire ----
            kts = []
            qtts = [qtp.tile([P, NSP, TCH, P], BF16, name="qtt_t", tag="qt_t")
                    for _ in range(CT)]
            for isp in range(NSP):
                kt_t = ktp.tile([P, TCH, C], BF16, name="kt_t")
                hf = TCH // 2
                for lo, hi in ((0, hf), (hf, TCH)):
                    nc.sync.dma_start(out=kt_t[:, lo:hi, :],
                                      in_=kt_ext[b, :, (NREG + isp) * TCH + lo:
                                                 (NREG + isp) * TCH + hi, :])
                nc.sync.dma_start(out=qtts[0][:, isp],
                                  in_=qtt_ext[b, 0, :, isp])
                kts.append(kt_t)
            nc.sync.dma_start(out=qtts[1][:, 0], in_=qtt_ext[b, 1, :, 0])
            nc.sync.dma_start(out=qtts[1][:, 1], in_=qtt_ext[b, 1, :, 1])

            # previous batch's stores: behind this batch's loads on the
            # sync ring, so they drain without contending with them
            for dst_ap, src_t in deferred_stores:
                nc.sync.dma_start(out=dst_ap, in_=src_t)
            deferred_stores = []

            for isp in range(NSP):
                for tl in range(TCH):
                    k2 = isp * TCH + tl
                    if b == 0 and k2 in WARM_SP_MM:
                        warm(WARM_SP_MM[k2])
                    if tl % 2 == 0:
                        drain(1)
                    t = (NREG + isp) * TCH + tl
                    nc.tensor.matmul(
                        eng[0],
                        lhsT=qtts[0][:, isp, tl, :],
                        rhs=kts[isp][:, tl, :],
                        start=False,
                        stop=(t == NT - 1),
                    )
                pending.extend((kts[isp], NREG + isp, g) for g in range(NG))

            # ---- softmax(max-e) = exp(min-e)/sum with beta/sum folded in
            # and I added to the diagonal block (residual fold) ----
            attnT = [None] * CT

            def emit_softmax(ci):
                mn = statp.tile([P, 1], F32)
                nc.vector.tensor_reduce(out=mn, in_=eng[ci],
                                        axis=mybir.AxisListType.X,
                                        op=mybir.AluOpType.min)
                ex = expp.tile([P, C], F32)
                sm = statp.tile([P, 1], F32)
                nc.scalar.activation(out=ex, in_=eng[ci],
                                     func=mybir.ActivationFunctionType.Exp,
                                     bias=mn, scale=-1.0, accum_out=sm)
                rc = statp.tile([P, 1], F32)
                nc.vector.reciprocal(out=rc, in_=sm)
                rb = statp.tile([P, 1], F32)
                nc.vector.tensor_mul(out=rb, in0=rc, in1=beta_sb)
                at = attp.tile([P, C], BF16)
                nc.vector.tensor_scalar_mul(out=at, in0=ex, scalar1=rb)
                nc.vector.tensor_add(out=at[:, ci * P:(ci + 1) * P],
                                     in0=at[:, ci * P:(ci + 1) * P],
                                     in1=ident)
                trA = trp.tile([P, CT, P], BF16, name="trA", tag="tr4")
                for dj in range(CT):
                    nc.tensor.transpose(trA[:, dj, :],
                                        at[:, dj * P:(dj + 1) * P], ident)
                atT = atTp.tile([P, CT, P], BF16, name="atT")
                nc.vector.tensor_copy(out=atT, in_=trA)
                attnT[ci] = atT

            # ci=1 energy over chunks 2..3; softmax(0)'s PE transposes slot
            # in at j=32 (~3.4us of chain-hiding cover); remaining natural-k
            # transpose groups drain in the gaps between matmul packs
            ci1_mms = [(isp, tl) for isp in range(NSP) for tl in range(TCH)]
            for j, (isp, tl) in enumerate(ci1_mms):
                if b == 0 and j in WARM_CI1:
                    warm(WARM_CI1[j])
                if j % 2 == 0:
                    drain(1)
                if j == 32:
                    emit_softmax(0)
                t = (NREG + isp) * TCH + tl
                nc.tensor.matmul(
                    eng[1],
                    lhsT=qtts[1][:, isp, tl, :],
                    rhs=kts[isp][:, tl, :],
                    start=False,
                    stop=(t == NT - 1),
                )
            drain(len(pending))

            # ---- out = (beta*A + I) @ k: psum holds the final values;
            # downcast copies alternate DVE/Activation; stores stream per
            # 1536-wide piece;  softmax(1) hides under the ci=0 slabs ----
            def emit_mm2_slab(ci, h2, softmax1_at=None):
                ot = outbp.tile([P, NH], BF16, name="ot")
                for w in range(WPH):
                    if w == softmax1_at:
                        emit_softmax(1)
                    ps = outp.tile([P, OW], F32, name="ps", tag="ps")
                    for dj in range(CT):
                        nc.tensor.matmul(
                            ps,
                            lhsT=attnT[ci][:, dj, :],
                            rhs=kb[dj, h2][:, w * OW:(w + 1) * OW],
                            start=(dj == 0),
                            stop=(dj == CT - 1),
                        )
                    if w % 2 == 0:
                        nc.vector.tensor_copy(
                            out=ot[:, w * OW:(w + 1) * OW], in_=ps)
                    else:
                        nc.scalar.copy(
                            out=ot[:, w * OW:(w + 1) * OW], in_=ps)
                    last_slab = b == BL - 1 and ci == CT - 1 and h2 == 1
                    if last_slab:
                        # 2-chunk pieces at the very end: the store queue
                        # stays non-empty so DMA init latencies hide, and
                        # the final piece is small
                        edges = [2, 4, 6, 7, 8, 9]
                        if w + 1 in edges:
                            lo = ([0] + edges)[edges.index(w + 1)] * OW
                            nc.sync.dma_start(
                                out=out_ext[b, ci * P:(ci + 1) * P,
                                            h2 * NH + lo:
                                            h2 * NH + (w + 1) * OW],
                                in_=ot[:, lo:(w + 1) * OW])
                    elif w % 3 == 2:
                        seg = w // 3
                        dst = out_ext[b, ci * P:(ci + 1) * P,
                                      h2 * NH + seg * SW:
                                      h2 * NH + (seg + 1) * SW]
                        src = ot[:, seg * SW:(seg + 1) * SW]
                        if b < BL - 1:
                            deferred_stores.append((dst, src))
                        else:
                            nc.sync.dma_start(out=dst, in_=src)

            emit_mm2_slab(0, 0)
            emit_mm2_slab(0, 1, softmax1_at=4)
            emit_mm2_slab(1, 0)
            emit_mm2_slab(1, 1)

    nc.compile()
    return nc


_NC_CACHE = None


def _get_program():
    global _NC_CACHE
    if _NC_CACHE is None:
        _NC_CACHE = _build_program()
    return _NC_CACHE


def pack_pm(Z):
    # bf16 partition-major: out[b, p, t, c] = Z[b, c, t*128+p]
    x = Z.reshape(B, C, NT, P).astype(ml_dtypes.bfloat16)
    return x.transpose(0, 3, 2, 1)


def kernel(Z1, Zr, beta):
    Z1 = np.asarray(Z1, dtype=np.float32)
    Zr = np.asarray(Zr, dtype=np.float32)
    beta = np.asarray(beta, dtype=np.float32).reshape(1)

    qt_full = pack_pm(Z1)
    qta = np.ascontiguousarray(qt_full[:, :, :TQT, :])
    qtb = np.ascontiguousarray(
        qt_full[:, :, TQT:, :].reshape(B, P, NSP, TCH, CT, P)
        .transpose(0, 4, 1, 2, 3, 5))
    kt = np.ascontiguousarray(pack_pm(Zr))

    in_maps = []
    for i in range(NCORES):
        s = slice(i * BL, (i + 1) * BL)
        in_maps.append({"qt": qta[s], "qtt": qtb[s], "kt": kt[s],
                        "beta": beta})

    nc = _get_program()
    res = run_bass_kernel_spmd(nc, in_maps, list(range(NCORES)))
    out = np.concatenate([r["out"] for r in res.results], axis=0)
    return out.astype(np.float32).reshape(B, C, H, W)


# revision 30
# speedup vs baseline: 1.0178x; 1.0073x over previous
"""Trainium2 Bass kernel for nn_Cross_Attention (B=16, C=256, H=W=96).

reference:
    q = Z1.reshape(B, C, N); k = Zr.reshape(B, C, N)         # N = H*W
    energy    = q @ k^T                                       # [B, C, C]
    attention = softmax(rowmax(energy) - energy, axis=-1)
    out       = attention @ k                                 # [B, C, N]
    return beta * out + Zr
ref absmax ~5.4, tol 2e-2 -> bf16 I/O rounding (~4e-3) is well inside it.

Strategy: data-parallel over batch, 2 batches per NeuronCore on 8 cores.
All HBM I/O is bf16 and minimal: q^T AND k^T are host-packed
partition-major (k^T replaces the natural-Zr upload -- same bytes), so
the N-contraction energy matmul streams BOTH operands straight from
DRAM with no on-chip preprocessing on its critical path.  Natural-layout
k (the mm2 rhs and, via the diagonal fold, the residual) is rebuilt
on-chip by TensorE transpose-mode matmuls whose PSUM->SBUF copies (one
Activation + one DVE copy per group) have an entire phase of slack --
they only feed mm2, ~15us later.  The transpose groups drain lazily,
interleaved a few matmuls at a time into the energy stream.

softmax(max-e) == exp(min-e)/sum(exp(min-e)): only a row-min is needed,
exp args are <= 0, the sum is >= 1.  The residual Zr IS k, so
beta*out + Zr == (beta*attention + I) @ k: beta and 1/sum fold into the
attention weights, I is added to their diagonal block, and mm2 produces
final values directly in PSUM (bitwise bf16(Zr) when beta == 0); its
psum->sbuf downcast copies alternate DVE/Activation.  q^T chunks 2-3 are
ci-split so eng[0] closes ~4us before eng[1], letting each softmax chain
hide under later matmul streams.  Batch b's stores queue on the sync
ring BEHIND batch b+1's loads so the store burst never steals DMA
bandwidth from the loads feeding the PE; the final slab stores in small
back-to-back pieces so DMA init latencies hide.  Dependency-free warm-up
matmuls bridge batch 0's unavoidable wire waits, because a drained PE
restarts at the 0.65GHz p-state for ~3us.
"""

from contextlib import ExitStack

import ml_dtypes
import numpy as np

import concourse.bass as bass
import concourse.tile as tile
from concourse import bacc, mybir
from concourse.bass_utils import run_bass_kernel_spmd
from concourse.masks import make_identity

B, C, H, W = 16, 256, 96, 96
N = H * W                    # 9216
P = 128
NCORES = 8
BL = B // NCORES             # 2 batches per core
CT = C // P                  # 2 c-tiles of 128
NT = N // P                  # 72 contraction tiles for energy
TCH = 18                     # tiles per chunk
NCH = NT // TCH              # 4 chunks (last two ci-split on the q side)
NREG = 2                     # chunks 0..1 are ci-interleaved
TQT = NREG * TCH             # 36 t-tiles in the interleaved qt tensor
NSP = NCH - NREG             # 2 ci-split chunks
NH = N // 2                  # 4608: kb slice width (half a c-tile row)
TPH = NH // P                # 36 n-tiles per h-half
OW = 512                     # mm2 psum chunk width == one full PSUM bank
WPH = NH // OW               # 9 psum chunks per h-half
SW = 3 * OW                  # 1536: store piece width (3 per h-half)

F32 = mybir.dt.float32
BF16 = mybir.dt.bfloat16


def _build_program():
    nc = bacc.Bacc("TRN2", target_bir_lowering=False, debug=False,
                   num_devices=NCORES)

    qt_ext = nc.dram_tensor("qt", [BL, P, TQT, C], BF16, kind="ExternalInput")
    qtt_ext = nc.dram_tensor("qtt", [BL, CT, P, NSP, TCH, P], BF16,
                             kind="ExternalInput")
    kt_ext = nc.dram_tensor("kt", [BL, P, NT, C], BF16, kind="ExternalInput")
    beta_ext = nc.dram_tensor("beta", [1], F32, kind="ExternalInput")
    out_ext = nc.dram_tensor("out", [BL, C, N], BF16, kind="ExternalOutput")

    with tile.TileContext(nc) as tc, ExitStack() as ctx:
        qtp = ctx.enter_context(tc.tile_pool(name="qtp", bufs=4))
        ktp = ctx.enter_context(tc.tile_pool(name="ktp", bufs=5))
        kbp = ctx.enter_context(tc.tile_pool(name="kbp", bufs=6))
        expp = ctx.enter_context(tc.tile_pool(name="expp", bufs=2))
        attp = ctx.enter_context(tc.tile_pool(name="attp", bufs=2))
        atTp = ctx.enter_context(tc.tile_pool(name="atTp", bufs=2))
        outbp = ctx.enter_context(tc.tile_pool(name="outbp", bufs=5))
        statp = ctx.enter_context(tc.tile_pool(name="statp", bufs=8))
        singles = ctx.enter_context(tc.tile_pool(name="singles", bufs=1))
        engp = ctx.enter_context(tc.tile_pool(name="engp", bufs=2, space="PSUM"))
        trp = ctx.enter_context(tc.tile_pool(name="trp", bufs=3, space="PSUM"))
        outp = ctx.enter_context(tc.tile_pool(name="outp", bufs=3, space="PSUM"))

        ident = singles.tile([P, P], BF16)
        beta_sb = singles.tile([P, 1], F32)
        make_identity(nc, ident)
        # after make_identity: the gpsimd ring's ~1us descriptor generation
        # must not sit ahead of the identity fill in the Pool queue
        nc.gpsimd.dma_start(out=beta_sb, in_=beta_ext.ap().to_broadcast((P, 1)))

        # warm-filler sizes for batch 0's wire waits (tuned from traces)
        WARM_HEAD = 28
        WARM_CH1 = 14
        WARM_SP_MM = {0: 12, 18: 16}     # before ch2-ci0 / ch3-ci0 mm streams
        WARM_CI1 = {0: 14, 18: 8}       # before ci1 mm j-indices

        GROUPS = [list(range(g * 4, min(g * 4 + 4, TCH)))
                  for g in range((TCH + 3) // 4)]
        NG = len(GROUPS)

        def warm(n):
            ps = outp.tile([P, OW], F32, name="warm", tag="ps")
            for _ in range(n):
                nc.tensor.matmul(ps[:, :P], lhsT=ident, rhs=ident,
                                 start=True, stop=True)

        def warmb(n):
            # mm2-phase filler: scratch transposes into the (idle) trp pool
            # -- outp rotation is what these fills are bridging, so the
            # energy-phase warm() scratch must not be used here
            trb = trp.tile([P, CT, 4, P], BF16, name="warmb", tag="tr4")
            for _ in range(n):
                nc.tensor.transpose(trb[:, 0, 0, :], ident, ident)

        def emit_tr_group(kt_t, i, g, kb):
            # rebuild natural-k for chunk i, tls GROUPS[g]: transpose each
            # [128n x 128d] sub-tile of the resident k^T chunk, then one
            # Activation copy (dj=0) + one DVE copy (dj=1) into kb
            h = i // 2
            tls = GROUPS[g]
            n = len(tls)
            th0 = i * TCH + tls[0] - h * TPH
            tr = trp.tile([P, CT, 4, P], BF16, name="tr4")
            for j, tl in enumerate(tls):
                for dj in range(CT):
                    nc.tensor.transpose(tr[:, dj, j, :],
                                        kt_t[:, tl, dj * P:(dj + 1) * P],
                                        ident)
            nc.scalar.copy(out=kb[0, h][:, th0 * P:(th0 + n) * P],
                           in_=tr[:, 0, :n, :])
            nc.vector.tensor_copy(out=kb[1, h][:, th0 * P:(th0 + n) * P],
                                  in_=tr[:, 1, :n, :])

        deferred_stores = []
        for b in range(BL):
            eng = [engp.tile([P, C], F32, name="eng") for _ in range(CT)]
            kb = {(cj, hh): kbp.tile([P, NH], BF16, name="kb_t")
                  for hh in range(2) for cj in range(CT)}
            pending = []          # (kt_t, i, g) transpose groups to drain

            def drain(nmax=1):
                for _ in range(min(nmax, len(pending))):
                    emit_tr_group(*pending.pop(0), kb)

            # ---- chunks 0..1, ci-interleaved q; energy matmuls consume
            # q^T and k^T straight from DRAM; previous chunk's transpose
            # groups drain between matmul packs ----
            for i in range(NREG):
                kt_t = ktp.tile([P, TCH, C], BF16, name="kt_t")
                qt_t = qtp.tile([P, TCH, C], BF16, name="qt_t")
                hf = TCH // 2
                # kernel head: extra-fine first pieces so the PE starts
                # its first real matmuls ~1.5us sooner
                pieces = ((0, 3), (3, hf), (hf, TCH)) if i == 0 and b == 0                     else ((0, hf), (hf, TCH))
                for lo, hi in pieces:
                    nc.sync.dma_start(out=kt_t[:, lo:hi, :],
                                      in_=kt_ext[b, :, i * TCH + lo:
                                                 i * TCH + hi, :])
                    nc.sync.dma_start(out=qt_t[:, lo:hi, :],
                                      in_=qt_ext[b, :, i * TCH + lo:
                                                 i * TCH + hi, :])
                if b == 0:
                    warm(WARM_HEAD if i == 0 else WARM_CH1)
                for tl in range(TCH):
                    if tl % 2 == 0:
                        drain(1)
                    t = i * TCH + tl
                    for ci in range(CT):
                        nc.tensor.matmul(
                            eng[ci],
                            lhsT=qt_t[:, tl, ci * P:(ci + 1) * P],
                            rhs=kt_t[:, tl, :],
                            start=(t == 0),
                            stop=False,
                        )
                pending.extend((kt_t, i, g) for g in range(NG))

            # ---- chunks 2..3: q ci-split so eng[0] closes early; k^T
            # chunks interleave with the qtt halves on the wire ----
            kts = []
            qtts = [qtp.tile([P, NSP, TCH, P], BF16, name="qtt_t", tag="qt_t")
                    for _ in range(CT)]
            for isp in range(NSP):
                kt_t = ktp.tile([P, TCH, C], BF16, name="kt_t")
                hf = TCH // 2
                for lo, hi in ((0, hf), (hf, TCH)):
                    nc.sync.dma_start(out=kt_t[:, lo:hi, :],
                                      in_=kt_ext[b, :, (NREG + isp) * TCH + lo:
                                                 (NREG + isp) * TCH + hi, :])
                nc.sync.dma_start(out=qtts[0][:, isp],
                                  in_=qtt_ext[b, 0, :, isp])
                kts.append(kt_t)
            nc.sync.dma_start(out=qtts[1][:, 0], in_=qtt_ext[b, 1, :, 0])
            nc.sync.dma_start(out=qtts[1][:, 1], in_=qtt_ext[b, 1, :, 1])

            # previous batch's stores: behind this batch's loads on the
            # sync ring, so they drain without contending with them
            for dst_ap, src_t in deferred_stores:
                nc.sync.dma_start(out=dst_ap, in_=src_t)
            deferred_stores = []

            for isp in range(NSP):
                for tl in range(TCH):
                    k2 = isp * TCH + tl
                    if b == 0 and k2 in WARM_SP_MM:
                        warm(WARM_SP_MM[k2])
                    if tl % 2 == 0:
                        drain(1)
                    t = (NREG + isp) * TCH + tl
                    nc.tensor.matmul(
                        eng[0],
                        lhsT=qtts[0][:, isp, tl, :],
                        rhs=kts[isp][:, tl, :],
                        start=False,
                        stop=(t == NT - 1),
                    )
                pending.extend((kts[isp], NREG + isp, g) for g in range(NG))

            # ---- softmax(max-e) = exp(min-e)/sum with beta/sum folded in
            # and I added to the diagonal block (residual fold) ----
            attnT = [None] * CT

            def emit_softmax(ci):
                mn = statp.tile([P, 1], F32)
                nc.vector.tensor_reduce(out=mn, in_=eng[ci],
                                        axis=mybir.AxisListType.X,
                                        op=mybir.AluOpType.min)
                ex = expp.tile([P, C], F32)
                sm = statp.tile([P, 1], F32)
                nc.scalar.activation(out=ex, in_=eng[ci],
                                     func=mybir.ActivationFunctionType.Exp,
                                     bias=mn, scale=-1.0, accum_out=sm)
                rc = statp.tile([P, 1], F32)
                nc.vector.reciprocal(out=rc, in_=sm)
                rb = statp.tile([P, 1], F32)
                nc.vector.tensor_mul(out=rb, in0=rc, in1=beta_sb)
                at = attp.tile([P, C], BF16)
                nc.vector.tensor_scalar_mul(out=at, in0=ex, scalar1=rb)
                nc.vector.tensor_add(out=at[:, ci * P:(ci + 1) * P],
                                     in0=at[:, ci * P:(ci + 1) * P],
                                     in1=ident)
                trA = trp.tile([P, CT, P], BF16, name="trA", tag="tr4")
                for dj in range(CT):
                    nc.tensor.transpose(trA[:, dj, :],
                                        at[:, dj * P:(dj + 1) * P], ident)
                atT = atTp.tile([P, CT, P], BF16, name="atT")
                nc.vector.tensor_copy(out=atT, in_=trA)
                attnT[ci] = atT

            # ci=1 energy over chunks 2..3; softmax(0)'s PE transposes slot
            # in at j=32 (~3.4us of chain-hiding cover); remaining natural-k
            # transpose groups drain in the gaps between matmul packs
            ci1_mms = [(isp, tl) for isp in range(NSP) for tl in range(TCH)]
            for j, (isp, tl) in enumerate(ci1_mms):
                if b == 0 and j in WARM_CI1:
                    warm(WARM_CI1[j])
                if j % 2 == 0:
                    drain(1)
                if j == 32:
                    emit_softmax(0)
                t = (NREG + isp) * TCH + tl
                nc.tensor.matmul(
                    eng[1],
                    lhsT=qtts[1][:, isp, tl, :],
                    rhs=kts[isp][:, tl, :],
                    start=False,
                    stop=(t == NT - 1),
                )
            drain(len(pending))

            # ---- out = (beta*A + I) @ k: psum holds the final values;
            # downcast copies alternate DVE/Activation; stores stream per
            # 1536-wide piece;  softmax(1) hides under the ci=0 slabs ----
            def emit_mm2_slab(ci, h2, softmax1_at=None):
                ot = outbp.tile([P, NH], BF16, name="ot")
                for w in range(WPH):
                    if w == softmax1_at:
                        emit_softmax(1)
                    ps = outp.tile([P, OW], F32, name="ps", tag="ps")
                    for dj in range(CT):
                        nc.tensor.matmul(
                            ps,
                            lhsT=attnT[ci][:, dj, :],
                            rhs=kb[dj, h2][:, w * OW:(w + 1) * OW],
                            start=(dj == 0),
                            stop=(dj == CT - 1),
                        )
                    if w % 2 == 0:
                        nc.vector.tensor_copy(
                            out=ot[:, w * OW:(w + 1) * OW], in_=ps)
                    else:
                        nc.scalar.copy(
                            out=ot[:, w * OW:(w + 1) * OW], in_=ps)
                    last_slab = b == BL - 1 and ci == CT - 1 and h2 == 1
                    if last_slab:
                        # 2-chunk pieces at the very end: the store queue
                        # stays non-empty so DMA init latencies hide, and
                        # the final piece is small
                        edges = [2, 4, 6, 7, 8, 9]
                        if w + 1 in edges:
                            lo = ([0] + edges)[edges.index(w + 1)] * OW
                            nc.sync.dma_start(
                                out=out_ext[b, ci * P:(ci + 1) * P,
                                            h2 * NH + lo:
                                            h2 * NH + (w + 1) * OW],
                                in_=ot[:, lo:(w + 1) * OW])
                    elif w % 3 == 2:
                        seg = w // 3
                        dst = out_ext[b, ci * P:(ci + 1) * P,
                                      h2 * NH + seg * SW:
                                      h2 * NH + (seg + 1) * SW]
                        src = ot[:, seg * SW:(seg + 1) * SW]
                        if b < BL - 1:
                            deferred_stores.append((dst, src))
                        else:
                            nc.sync.dma_start(out=dst, in_=src)

            emit_mm2_slab(0, 0)
            emit_mm2_slab(0, 1, softmax1_at=4)
            emit_mm2_slab(1, 0)
            emit_mm2_slab(1, 1)

    nc.compile()
    return nc


_NC_CACHE = None


def _get_program():
    global _NC_CACHE
    if _NC_CACHE is None:
        _NC_CACHE = _build_program()
    return _NC_CACHE


def pack_pm(Z):
    # bf16 partition-major: out[b, p, t, c] = Z[b, c, t*128+p]
    x = Z.reshape(B, C, NT, P).astype(ml_dtypes.bfloat16)
    return x.transpose(0, 3, 2, 1)


def kernel(Z1, Zr, beta):
    Z1 = np.asarray(Z1, dtype=np.float32)
    Zr = np.asarray(Zr, dtype=np.float32)
    beta = np.asarray(beta, dtype=np.float32).reshape(1)

    qt_full = pack_pm(Z1)
    qta = np.ascontiguousarray(qt_full[:, :, :TQT, :])
    qtb = np.ascontiguousarray(
        qt_full[:, :, TQT:, :].reshape(B, P, NSP, TCH, CT, P)
        .transpose(0, 4, 1, 2, 3, 5))
    kt = np.ascontiguousarray(pack_pm(Zr))

    in_maps = []
    for i in range(NCORES):
        s = slice(i * BL, (i + 1) * BL)
        in_maps.append({"qt": qta[s], "qtt": qtb[s], "kt": kt[s],
                        "beta": beta})

    nc = _get_program()
    res = run_bass_kernel_spmd(nc, in_maps, list(range(NCORES)))
    out = np.concatenate([r["out"] for r in res.results], axis=0)
    return out.astype(np.float32).reshape(B, C, H, W)
